# revision 39
# baseline (speedup 1.0000x reference)
"""Masked-MLP (CorticalColumnMLP) Trainium2 kernel.

Math: out = gelu(x @ (w1*mask1).T, exact) @ (w2*mask2).T

Key structural fact: mask1 zeroes whole rows of w1 and mask2 zeroes whole
columns of w2 (2-of-4 structured sparsity). gelu(0) == 0, so only hidden
units j with mask1-row j kept AND mask2-col j kept contribute to the
output. Sharding therefore selects exactly those hidden units: the device
runs a dense MLP over the ~2065 surviving hidden units (padded to a
multiple of 128 with zeros, which is exact).

Distribution: pure data-parallel over tokens. Each of the 8 cores gets
1/8 of the 8192 tokens and the full (gathered) weight set; outputs are
disjoint token slices, concatenated on the host. No collectives.

Device layout (per core): everything is laid out so every DMA is
contiguous per partition and no transposes are needed on device:
  xt   [128, KD, Tc]   xt[p,k,t] = x[t, k*128+p]         (lhs, K=D on partitions)
  w1d  [KD, 128, JT, 128]  w1d[k,p,j,c] = W1g[j*128+c, k*128+p]  (k-major slabs)
  w2d  [NT, 128, JT, 128]  w2d[n,p,j,c] = W2g.T[j*128+p, n*128+c]
  outt [NT, 128, Tc]   outt[n,p,t] = out[t, n*128+p]
Layer 1 computes hT[j,t] (hidden-major) so layer 2 can contract over
hidden without any transpose.

v2: w1 streams in k-major slabs of a j-wave ([128, JW, 128] per (wave,k))
so the first matmul only waits for slab (w0,k0) + x chunk k0 (~0.4 MB)
instead of the whole first wave of w1 (2 MB); warmup shrinks accordingly.
The final output tile is processed in 128-column chunks so the tail
copy+DMA after the last matmul is ~4x smaller.
"""

import os

import numpy as np
import ml_dtypes

import concourse.bass as bass
import concourse.mybir as mybir
import concourse.tile as tile
from concourse import bacc
from concourse.bass import ts
from concourse.bass_utils import run_bass_kernel_spmd

P = 128
TS = 512  # matmul moving free dim / PSUM bank width (fp32)
N_CORES = 8

# "bf16" | "f16" | "f32r" | "f32" — device matmul dtype
MM_DTYPE = os.environ.get("BASS_MLP_DTYPE", "f16")
WARMUP = int(os.environ.get("BASS_MLP_WARMUP", "22"))

_DT = {
    "bf16": mybir.dt.bfloat16,
    "f16": mybir.dt.float16,
    "f32r": mybir.dt.float32r,
    "f32": mybir.dt.float32,
}
_NPDT = {
    "bf16": ml_dtypes.bfloat16,
    "f16": np.float16,
    "f32r": np.float32,
    "f32": np.float32,
}

# result of the last run_bass_kernel_spmd call (for test harness inspection)
LAST_RESULT = None

_NC_CACHE = {}


def _build_nc(D, Hg, Tc, mode, act="Gelu"):
    """Build + compile the per-core Bass program (dense MLP, hidden=Hg)."""
    act_fn = getattr(mybir.ActivationFunctionType, act)
    dt_in = _DT[mode]
    f32 = mybir.dt.float32
    KD = D // P
    JT = Hg // P
    NT = D // P
    nTS = Tc // TS
    assert D % P == 0 and Hg % P == 0 and Tc % TS == 0

    # phase-A wave width (j-tiles processed k-major together); JW*nTS PSUM
    # groups are in flight at once (PSUM has 8 banks).
    four_byte = mode in ("f32r", "f32")
    JW = 3 if four_byte else 4
    w1_bufs = KD  # all k-slices of w1 SBUF-resident
    w2_bufs = 2 if four_byte else 4
    o_bufs = 2 if four_byte else 4

    waves = [list(range(w0, min(w0 + JW, JT))) for w0 in range(0, JT, JW)]

    NW = len(waves)
    nc = bacc.Bacc("TRN2", target_bir_lowering=False, debug=False,
                   num_devices=N_CORES)
    xt = nc.dram_tensor("xt", [P, KD, Tc], dt_in, kind="ExternalInput")
    w1d = nc.dram_tensor("w1d", [P, NW * KD * JW * P], dt_in,
                         kind="ExternalInput")
    w2d = nc.dram_tensor("w2d", [NT, P, JT, P], dt_in, kind="ExternalInput")
    outt = nc.dram_tensor("outt", [NT, P, Tc], dt_in, kind="ExternalOutput")

    with tile.TileContext(nc) as tc:
        with (
            tc.tile_pool(name="xp", bufs=1) as xp,
            tc.tile_pool(name="w1p", bufs=w1_bufs) as w1p,
            tc.tile_pool(name="w2p", bufs=w2_bufs) as w2p,
            tc.tile_pool(name="hp", bufs=1) as hp,
            tc.tile_pool(name="op", bufs=o_bufs) as op,
            tc.tile_pool(name="pp", bufs=8, space="PSUM") as pp,
            tc.tile_pool(name="wup", bufs=1) as wup,
        ):
            # PE warm-up: trivial matmuls on scratch data keep the PE busy
            # while the first w1 slab + x chunk land (~3-4us) and open the
            # HAM clock gate. fp32 tile: fp16/bf16 MEMSET is not a valid
            # ISA instruction; fp32 matmuls span the wait in few instrs.
            wu = wup.tile([P, P], f32)
            nc.vector.memset(wu, 0.0)
            wups = pp.tile([P, TS], f32, tag="ps", name="warm_ps")
            for _ in range(WARMUP):
                nc.tensor.matmul(wups[:, :P], lhsT=wu, rhs=wu,
                                 start=True, stop=True)

            x_tile = xp.tile([P, KD, Tc], dt_in)
            hT = hp.tile([P, JT, Tc], dt_in)

            # Layer 1: hT[j_tile, t] = gelu(sum_k w1.T @ x), in waves of JW
            # j-tiles (PSUM: JW*nTS = 8 banks in flight), k-major.
            # DMA: w1 arrives as 16 whole k-slices [P, JT, P] on the scalar
            # queue. The DMA engines move ~one per-partition-row packet per
            # 170ns per engine, so wide rows (JT*P*2 = 4.3KB here) deliver
            # ~4x faster than per-wave slabs would; the whole w1 lands in
            # ~12us and stays SBUF-resident, so no wave ever waits on
            # weights. x streams k-chunks on the sync queue in step.
            # x chunk widths: two singles to get k0/k1 landed fast, then
            # k-pairs (4KB contiguous rows -> full packet efficiency). x
            # alternates between the sync and gpsimd queues so it holds
            # ~2/3 of the DMA packet slots during wave 0, the window where
            # combined x+w1 demand approaches the DMA fabric cap.
            xch = []
            i = 0
            while i < KD:
                w = 1 if (i < 2 and KD - i > 2) else min(2, KD - i)
                xch.append((i, w))
                i += w
            for xi, (i0, w) in enumerate(xch):
                x_eng = nc.sync if xi % 2 == 0 else nc.gpsimd
                x_eng.dma_start(x_tile[:, i0:i0 + w, :], xt[:, i0:i0 + w, :])

            # w1 arrives per (wave, k-group) as flat slabs whose rows are
            # KK*JW*P contiguous: wave 0 uses doubling group sizes (2,2,4,
            # 8..) so k0 lands fast while later fetches ride at full packet
            # efficiency; waves >=1 are one whole-wave fetch each (16KB
            # rows). Every slab has its own buffer (bufs=1, all of w1 stays
            # SBUF-resident) so the scalar queue streams them back-to-back
            # with zero reuse waits, spread across L1's whole timeline.
            def groups(first):
                # Quarters for every wave: rows ~4KB — big enough for full
                # packet efficiency, small enough that per-packet
                # round-robin arbitration stays byte-fair with the x and w2
                # streams (16KB rows were observed to crowd out x 4:1).
                gs, i, sz = [], 0, max(1, KD // 4)
                while i < KD:
                    g = min(sz, KD - i)
                    gs.append((i, g))
                    i += g
                return gs

            w1ts = {}
            for wi in range(len(waves)):
                gs = groups(wi == 0)
                for gi, (k0, KK) in enumerate(gs):
                    slab = w1p.tile([P, KK * JW * P], dt_in, bufs=1,
                                    tag=f"w1s_{wi}_{k0}",
                                    name=f"w1s_{wi}_{k0}")
                    w1ts[wi, k0] = (slab, KK)
                    off = (wi * KD + k0) * JW * P
                    # wave 0's last quarter rides the gpsimd queue (idle
                    # after its x chunks) — the scalar queue alone is
                    # marginal against wave 0's k12-15 deadline.
                    w_eng = (nc.gpsimd if wi == 0 and gi == len(gs) - 1
                             else nc.scalar)
                    w_eng.dma_start(slab, w1d[:, off:off + KK * JW * P])

            w2ts = {}
            for wi, js in enumerate(waves):
                if wi == len(waves) - 1:
                    # w2 head start; sits behind all of w1 in the scalar
                    # queue FIFO, far from the critical window either way.
                    for n in range(min(w2_bufs, NT)):
                        w2ts[n] = w2p.tile([P, JT, P], dt_in, tag="w2",
                                           name=f"w2t{n}")
                        nc.scalar.dma_start(w2ts[n], w2d[n])
                pss = {}
                for j in js:
                    for t in range(nTS):
                        pss[j, t] = pp.tile([P, TS], f32, tag="ps",
                                            name=f"psA{j}_{t}")
                gmap = {}
                for (k0, KK) in groups(wi == 0):
                    for kk in range(KK):
                        gmap[k0 + kk] = (w1ts[wi, k0][0], kk)
                for k in range(KD):
                    slab, kk = gmap[k]
                    for jj, j in enumerate(js):
                        o0 = (kk * JW + jj) * P
                        for t in range(nTS):
                            nc.tensor.matmul(
                                pss[j, t], lhsT=slab[:, o0:o0 + P],
                                rhs=x_tile[:, k, ts(t, TS)],
                                start=(k == 0), stop=(k == KD - 1),
                            )
                for j in js:
                    for t in range(nTS):
                        nc.scalar.activation(hT[:, j, ts(t, TS)], pss[j, t],
                                             act_fn)

            # Layer 2: outT[n_tile, t] = sum_j w2g.T @ hT. j is the outer
            # loop so each w2 stationary tile serves both t-chunks
            # back-to-back (halves the weight reloads). Output is copied
            # to fp16 (2x DVE rate, half the DMA bytes); the final tile's
            # DMA is split so the tail after the last matmul is short.
            for n in range(NT):
                if n in w2ts:
                    w2t = w2ts[n]
                else:
                    w2t = w2p.tile([P, JT, P], dt_in, tag="w2")
                    nc.scalar.dma_start(w2t, w2d[n])
                pss = [pp.tile([P, TS], f32, tag="ps", name=f"psB{n}_{t}")
                       for t in range(nTS)]
                if n < NT - 1:
                    # j-outer: each w2 stationary tile serves all t-chunks.
                    for j in range(JT):
                        for t in range(nTS):
                            nc.tensor.matmul(
                                pss[t], lhsT=w2t[:, j, :],
                                rhs=hT[:, j, ts(t, TS)],
                                start=(j == 0), stop=(j == JT - 1),
                            )
                    for t in range(nTS):
                        ot = op.tile([P, TS], dt_in, tag="o")
                        nc.vector.tensor_copy(ot, pss[t])
                        nc.sync.dma_start(outt[n, :, ts(t, TS)], ot)
                else:
                    # Last tile: t-outer so t0's copy+DMA hide behind t1's
                    # matmuls and only one copy+DMA chain trails the stream.
                    for t in range(nTS):
                        for j in range(JT):
                            nc.tensor.matmul(
                                pss[t], lhsT=w2t[:, j, :],
                                rhs=hT[:, j, ts(t, TS)],
                                start=(j == 0), stop=(j == JT - 1),
                            )
                        ot = op.tile([P, TS], dt_in, tag="o")
                        nc.vector.tensor_copy(ot, pss[t])
                        nc.sync.dma_start(outt[n, :, ts(t, TS)], ot)

    nc.compile()
    return nc


def _get_nc(D, Hg, Tc, mode):
    key = (D, Hg, Tc, mode)
    if key not in _NC_CACHE:
        _NC_CACHE[key] = _build_nc(D, Hg, Tc, mode)
    return _NC_CACHE[key]


def _pack_w1(W1, Hg, JW, npdt):
    # W1 [Hk, D] -> rows padded to NW*JW*P -> flat per-wave k-major:
    # w1d[p, ((wi*KD + k)*JW + j)*P + c] = W1[(wi*JW + j)*P + c, k*P + p]
    Hk, D = W1.shape
    KD = D // P
    NW = (Hg // P + JW - 1) // JW
    W1p = np.zeros((NW * JW * P, D), np.float32)
    W1p[:Hk] = W1
    a = W1p.reshape(NW, JW, P, KD, P).transpose(4, 0, 3, 1, 2)
    return np.ascontiguousarray(a.reshape(P, NW * KD * JW * P)).astype(npdt)


def _pack_w2(W2T, Hg, npdt):
    # W2T [Hk, D] (= w2[:, kb].T) -> padded [Hg, D]
    # w2d[n,p,j,c] = W2T[j*P+p, n*P+c]
    Hk, D = W2T.shape
    W2p = np.zeros((Hg, D), np.float32)
    W2p[:Hk] = W2T
    a = W2p.reshape(Hg // P, P, D // P, P).transpose(2, 1, 0, 3)
    return np.ascontiguousarray(a).astype(npdt)


def _pack_x(xc, npdt):
    # xc [Tc, D] -> xt[p,k,t] = xc[t, k*P+p]
    Tc, D = xc.shape
    a = xc.T.reshape(D // P, P, Tc).transpose(1, 0, 2)
    return np.ascontiguousarray(a).astype(npdt)


def kernel(x, w1, w2, mask1, mask2, _trace=False):
    mode = MM_DTYPE
    npdt = _NPDT[mode]

    x = np.asarray(x, np.float32)
    w1 = np.asarray(w1, np.float32)
    w2 = np.asarray(w2, np.float32)
    mask1 = np.asarray(mask1, np.float32)
    mask2 = np.asarray(mask2, np.float32)

    B, S, D = x.shape
    T = B * S
    H = w1.shape[0]
    x2 = x.reshape(T, D)

    # Sharding of the hidden dimension: keep only hidden units whose
    # mask1 row and mask2 column are nonzero (the rest contribute exactly
    # zero). Requires whole-row / whole-column masks, which is what this
    # module's sparsity pattern guarantees; otherwise fall back to dense.
    structured = bool((mask1 == mask1[:, :1]).all()) and bool(
        (mask2 == mask2[:1, :]).all()
    )
    if structured:
        k1 = np.flatnonzero(mask1[:, 0])
        k2 = np.flatnonzero(mask2[0, :])
        kb = np.intersect1d(k1, k2)
        if kb.size == 0:
            return np.zeros((B, S, D), np.float32)
        W1 = w1[kb]             # [Hk, D], mask1 rows are all-ones here
        W2T = w2[:, kb].T       # [Hk, D], mask2 cols are all-ones here
    else:
        W1 = w1 * mask1
        W2T = (w2 * mask2).T
        if mode in ("f32r", "f32"):
            mode = "f16"        # dense fallback: halve SBUF footprint
            npdt = _NPDT[mode]
    Hk = W1.shape[0]
    Hg = max(P, ((Hk + P - 1) // P) * P)

    four_byte = mode in ("f32r", "f32")
    w1d = _pack_w1(W1, Hg, 3 if four_byte else 4, npdt)
    w2d = _pack_w2(W2T, Hg, npdt)

    # Token-parallel over cores, in sequential rounds if a full token
    # slice per core would not divide into TS chunks or not fit in SBUF
    # (x + hT are SBUF-resident: (KD + JT) * Tc * itemsize per partition).
    assert T % N_CORES == 0
    Tc = T // N_CORES
    itemsz = np.dtype(npdt).itemsize
    rounds = 1
    while (Tc // rounds) % TS != 0 or (Tc // rounds) == 0 or (
        (D // P + Hg // P) * (Tc // rounds) * itemsz > 140 * 1024
    ):
        rounds *= 2
        assert rounds <= 16, "input too large for SBUF tiling scheme"
    Tc //= rounds

    nc = _get_nc(D, Hg, Tc, mode)

    out = np.empty((T, D), np.float32)
    global LAST_RESULT
    for r in range(rounds):
        in_maps = []
        for c in range(N_CORES):
            t0 = (r * N_CORES + c) * Tc
            in_maps.append({
                "xt": _pack_x(x2[t0:t0 + Tc], npdt),
                "w1d": w1d,
                "w2d": w2d,
            })
        res = run_bass_kernel_spmd(
            nc, in_maps, core_ids=list(range(N_CORES)), trace=_trace,
        )
        LAST_RESULT = res
        for c in range(N_CORES):
            t0 = (r * N_CORES + c) * Tc
            o = res.results[c]["outt"]  # [NT, P, Tc] in device dtype
            out[t0:t0 + Tc] = o.reshape(D, Tc).T.astype(np.float32)

    return out.reshape(B, S, D)


# revision 40
# speedup vs baseline: 1.0510x; 1.0510x over previous
"""Masked-MLP (CorticalColumnMLP) Trainium2 kernel.

Math: out = gelu(x @ (w1*mask1).T, exact) @ (w2*mask2).T

Key structural fact: mask1 zeroes whole rows of w1 and mask2 zeroes whole
columns of w2 (2-of-4 structured sparsity). gelu(0) == 0, so only hidden
units j with mask1-row j kept AND mask2-col j kept contribute to the
output. Sharding therefore selects exactly those hidden units: the device
runs a dense MLP over the ~2065 surviving hidden units (padded to a
multiple of 128 with zeros, which is exact).

Distribution: pure data-parallel over tokens. Each of the 8 cores gets
1/8 of the 8192 tokens and the full (gathered) weight set; outputs are
disjoint token slices, concatenated on the host. No collectives.

Device layout (per core): everything is laid out so every DMA is
contiguous per partition and no transposes are needed on device:
  xt   [128, KD, Tc]   xt[p,k,t] = x[t, k*128+p]         (lhs, K=D on partitions)
  w1d  [KD, 128, JT, 128]  w1d[k,p,j,c] = W1g[j*128+c, k*128+p]  (k-major slabs)
  w2d  [NT, 128, JT, 128]  w2d[n,p,j,c] = W2g.T[j*128+p, n*128+c]
  outt [NT, 128, Tc]   outt[n,p,t] = out[t, n*128+p]
Layer 1 computes hT[j,t] (hidden-major) so layer 2 can contract over
hidden without any transpose.

v2: w1 streams in k-major slabs of a j-wave ([128, JW, 128] per (wave,k))
so the first matmul only waits for slab (w0,k0) + x chunk k0 (~0.4 MB)
instead of the whole first wave of w1 (2 MB); warmup shrinks accordingly.
The final output tile is processed in 128-column chunks so the tail
copy+DMA after the last matmul is ~4x smaller.
"""

import os

import numpy as np
import ml_dtypes

import concourse.bass as bass
import concourse.mybir as mybir
import concourse.tile as tile
from concourse import bacc
from concourse.bass import ts
from concourse.bass_utils import run_bass_kernel_spmd

P = 128
TS = 512  # matmul moving free dim / PSUM bank width (fp32)
N_CORES = 8

# "bf16" | "f16" | "f32r" | "f32" — device matmul dtype
MM_DTYPE = os.environ.get("BASS_MLP_DTYPE", "f16")
WARMUP = int(os.environ.get("BASS_MLP_WARMUP", "22"))

_DT = {
    "bf16": mybir.dt.bfloat16,
    "f16": mybir.dt.float16,
    "f32r": mybir.dt.float32r,
    "f32": mybir.dt.float32,
}
_NPDT = {
    "bf16": ml_dtypes.bfloat16,
    "f16": np.float16,
    "f32r": np.float32,
    "f32": np.float32,
}

# result of the last run_bass_kernel_spmd call (for test harness inspection)
LAST_RESULT = None

_NC_CACHE = {}


def _build_nc(D, Hg, Tc, mode, act="Gelu"):
    """Build + compile the per-core Bass program (dense MLP, hidden=Hg)."""
    act_fn = getattr(mybir.ActivationFunctionType, act)
    dt_in = _DT[mode]
    f32 = mybir.dt.float32
    KD = D // P
    JT = Hg // P
    NT = D // P
    nTS = Tc // TS
    assert D % P == 0 and Hg % P == 0 and Tc % TS == 0

    # phase-A wave width (j-tiles processed k-major together); JW*nTS PSUM
    # groups are in flight at once (PSUM has 8 banks).
    four_byte = mode in ("f32r", "f32")
    JW = 3 if four_byte else 4
    w1_bufs = KD  # all k-slices of w1 SBUF-resident
    w2_bufs = 2 if four_byte else 4
    o_bufs = 2 if four_byte else 4

    waves = [list(range(w0, min(w0 + JW, JT))) for w0 in range(0, JT, JW)]

    NW = len(waves)
    nc = bacc.Bacc("TRN2", target_bir_lowering=False, debug=False,
                   num_devices=N_CORES)
    xt = nc.dram_tensor("xt", [P, KD, Tc], dt_in, kind="ExternalInput")
    w1d = nc.dram_tensor("w1d", [P, NW * KD * JW * P], dt_in,
                         kind="ExternalInput")
    w2d = nc.dram_tensor("w2d", [NT, P, JT, P], dt_in, kind="ExternalInput")
    outt = nc.dram_tensor("outt", [NT, P, Tc], dt_in, kind="ExternalOutput")

    with tile.TileContext(nc) as tc:
        with (
            tc.tile_pool(name="xp", bufs=1) as xp,
            tc.tile_pool(name="w1p", bufs=w1_bufs) as w1p,
            tc.tile_pool(name="w2p", bufs=w2_bufs) as w2p,
            tc.tile_pool(name="hp", bufs=1) as hp,
            tc.tile_pool(name="op", bufs=o_bufs) as op,
            tc.tile_pool(name="pp", bufs=8, space="PSUM") as pp,
            tc.tile_pool(name="wup", bufs=1) as wup,
        ):
            # PE warm-up: trivial matmuls on scratch data keep the PE busy
            # while the first w1 slab + x chunk land (~3-4us) and open the
            # HAM clock gate. fp32 tile: fp16/bf16 MEMSET is not a valid
            # ISA instruction; fp32 matmuls span the wait in few instrs.
            wu = wup.tile([P, P], f32)
            nc.vector.memset(wu, 0.0)
            wups = pp.tile([P, TS], f32, tag="ps", name="warm_ps")
            for _ in range(WARMUP):
                nc.tensor.matmul(wups[:, :P], lhsT=wu, rhs=wu,
                                 start=True, stop=True)

            x_tile = xp.tile([P, KD, Tc], dt_in)
            hT = hp.tile([P, JT, Tc], dt_in)

            # Layer 1: hT[j_tile, t] = gelu(sum_k w1.T @ x), in waves of JW
            # j-tiles (PSUM: JW*nTS = 8 banks in flight), k-major.
            # DMA: w1 arrives as 16 whole k-slices [P, JT, P] on the scalar
            # queue. The DMA engines move ~one per-partition-row packet per
            # 170ns per engine, so wide rows (JT*P*2 = 4.3KB here) deliver
            # ~4x faster than per-wave slabs would; the whole w1 lands in
            # ~12us and stays SBUF-resident, so no wave ever waits on
            # weights. x streams k-chunks on the sync queue in step.
            # x chunk widths: two singles to get k0/k1 landed fast, then
            # k-pairs (4KB contiguous rows -> full packet efficiency). x
            # alternates between the sync and gpsimd queues so it holds
            # ~2/3 of the DMA packet slots during wave 0, the window where
            # combined x+w1 demand approaches the DMA fabric cap.
            xch = []
            i = 0
            while i < KD:
                w = 1 if (i < 2 and KD - i > 2) else min(2, KD - i)
                xch.append((i, w))
                i += w
            for xi, (i0, w) in enumerate(xch):
                x_eng = nc.sync if xi % 2 == 0 else nc.gpsimd
                x_eng.dma_start(x_tile[:, i0:i0 + w, :], xt[:, i0:i0 + w, :])

            # w1 arrives per (wave, k-group) as flat slabs whose rows are
            # KK*JW*P contiguous: wave 0 uses doubling group sizes (2,2,4,
            # 8..) so k0 lands fast while later fetches ride at full packet
            # efficiency; waves >=1 are one whole-wave fetch each (16KB
            # rows). Every slab has its own buffer (bufs=1, all of w1 stays
            # SBUF-resident) so the scalar queue streams them back-to-back
            # with zero reuse waits, spread across L1's whole timeline.
            def groups(first):
                # Quarters for every wave: rows ~4KB — big enough for full
                # packet efficiency, small enough that per-packet
                # round-robin arbitration stays byte-fair with the x and w2
                # streams (16KB rows were observed to crowd out x 4:1).
                gs, i, sz = [], 0, max(1, KD // 4)
                while i < KD:
                    g = min(sz, KD - i)
                    gs.append((i, g))
                    i += g
                return gs

            w1ts = {}
            for wi in range(len(waves)):
                gs = groups(wi == 0)
                for gi, (k0, KK) in enumerate(gs):
                    slab = w1p.tile([P, KK * JW * P], dt_in, bufs=1,
                                    tag=f"w1s_{wi}_{k0}",
                                    name=f"w1s_{wi}_{k0}")
                    w1ts[wi, k0] = (slab, KK)
                    off = (wi * KD + k0) * JW * P
                    # wave 0's last quarter and half of each later wave
                    # ride the gpsimd queue (idle once its x chunks are
                    # done) — the scalar queue alone is marginal against
                    # the wave 0 k12-15 and wave 1 deadlines.
                    if wi == 0:
                        w_eng = nc.gpsimd if gi == len(gs) - 1 else nc.scalar
                    else:
                        w_eng = nc.gpsimd if gi % 2 == 1 else nc.scalar
                    w_eng.dma_start(slab, w1d[:, off:off + KK * JW * P])

            w2ts = {}
            for wi, js in enumerate(waves):
                if wi == len(waves) - 1:
                    # w2 head start; sits behind all of w1 in the scalar
                    # queue FIFO, far from the critical window either way.
                    for n in range(min(w2_bufs, NT)):
                        w2ts[n] = w2p.tile([P, JT, P], dt_in, tag="w2",
                                           name=f"w2t{n}")
                        nc.scalar.dma_start(w2ts[n], w2d[n])
                pss = {}
                for j in js:
                    for t in range(nTS):
                        pss[j, t] = pp.tile([P, TS], f32, tag="ps",
                                            name=f"psA{j}_{t}")
                gmap = {}
                for (k0, KK) in groups(wi == 0):
                    for kk in range(KK):
                        gmap[k0 + kk] = (w1ts[wi, k0][0], kk)
                for k in range(KD):
                    slab, kk = gmap[k]
                    for jj, j in enumerate(js):
                        o0 = (kk * JW + jj) * P
                        for t in range(nTS):
                            nc.tensor.matmul(
                                pss[j, t], lhsT=slab[:, o0:o0 + P],
                                rhs=x_tile[:, k, ts(t, TS)],
                                start=(k == 0), stop=(k == KD - 1),
                            )
                for j in js:
                    for t in range(nTS):
                        nc.scalar.activation(hT[:, j, ts(t, TS)], pss[j, t],
                                             act_fn)

            # Layer 2: outT[n_tile, t] = sum_j w2g.T @ hT. j is the outer
            # loop so each w2 stationary tile serves both t-chunks
            # back-to-back (halves the weight reloads). Output is copied
            # to fp16 (2x DVE rate, half the DMA bytes); the final tile's
            # DMA is split so the tail after the last matmul is short.
            for n in range(NT):
                if n in w2ts:
                    w2t = w2ts[n]
                else:
                    w2t = w2p.tile([P, JT, P], dt_in, tag="w2")
                    nc.scalar.dma_start(w2t, w2d[n])
                pss = [pp.tile([P, TS], f32, tag="ps", name=f"psB{n}_{t}")
                       for t in range(nTS)]
                if n < NT - 1:
                    # j-outer: each w2 stationary tile serves all t-chunks.
                    for j in range(JT):
                        for t in range(nTS):
                            nc.tensor.matmul(
                                pss[t], lhsT=w2t[:, j, :],
                                rhs=hT[:, j, ts(t, TS)],
                                start=(j == 0), stop=(j == JT - 1),
                            )
                    for t in range(nTS):
                        ot = op.tile([P, TS], dt_in, tag="o")
                        nc.vector.tensor_copy(ot, pss[t])
                        nc.sync.dma_start(outt[n, :, ts(t, TS)], ot)
                else:
                    # Last tile: t-outer so t0's copy+DMA hide behind t1's
                    # matmuls and only one copy+DMA chain trails the stream.
                    for t in range(nTS):
                        for j in range(JT):
                            nc.tensor.matmul(
                                pss[t], lhsT=w2t[:, j, :],
                                rhs=hT[:, j, ts(t, TS)],
                                start=(j == 0), stop=(j == JT - 1),
                            )
                        ot = op.tile([P, TS], dt_in, tag="o")
                        nc.vector.tensor_copy(ot, pss[t])
                        nc.sync.dma_start(outt[n, :, ts(t, TS)], ot)

    nc.compile()
    return nc


def _get_nc(D, Hg, Tc, mode):
    key = (D, Hg, Tc, mode)
    if key not in _NC_CACHE:
        _NC_CACHE[key] = _build_nc(D, Hg, Tc, mode)
    return _NC_CACHE[key]


def _pack_w1(W1, Hg, JW, npdt):
    # W1 [Hk, D] -> rows padded to NW*JW*P -> flat per-wave k-major:
    # w1d[p, ((wi*KD + k)*JW + j)*P + c] = W1[(wi*JW + j)*P + c, k*P + p]
    Hk, D = W1.shape
    KD = D // P
    NW = (Hg // P + JW - 1) // JW
    W1p = np.zeros((NW * JW * P, D), np.float32)
    W1p[:Hk] = W1
    a = W1p.reshape(NW, JW, P, KD, P).transpose(4, 0, 3, 1, 2)
    return np.ascontiguousarray(a.reshape(P, NW * KD * JW * P)).astype(npdt)


def _pack_w2(W2T, Hg, npdt):
    # W2T [Hk, D] (= w2[:, kb].T) -> padded [Hg, D]
    # w2d[n,p,j,c] = W2T[j*P+p, n*P+c]
    Hk, D = W2T.shape
    W2p = np.zeros((Hg, D), np.float32)
    W2p[:Hk] = W2T
    a = W2p.reshape(Hg // P, P, D // P, P).transpose(2, 1, 0, 3)
    return np.ascontiguousarray(a).astype(npdt)


def _pack_x(xc, npdt):
    # xc [Tc, D] -> xt[p,k,t] = xc[t, k*P+p]
    Tc, D = xc.shape
    a = xc.T.reshape(D // P, P, Tc).transpose(1, 0, 2)
    return np.ascontiguousarray(a).astype(npdt)


def kernel(x, w1, w2, mask1, mask2, _trace=False):
    mode = MM_DTYPE
    npdt = _NPDT[mode]

    x = np.asarray(x, np.float32)
    w1 = np.asarray(w1, np.float32)
    w2 = np.asarray(w2, np.float32)
    mask1 = np.asarray(mask1, np.float32)
    mask2 = np.asarray(mask2, np.float32)

    B, S, D = x.shape
    T = B * S
    H = w1.shape[0]
    x2 = x.reshape(T, D)

    # Sharding of the hidden dimension: keep only hidden units whose
    # mask1 row and mask2 column are nonzero (the rest contribute exactly
    # zero). Requires whole-row / whole-column masks, which is what this
    # module's sparsity pattern guarantees; otherwise fall back to dense.
    structured = bool((mask1 == mask1[:, :1]).all()) and bool(
        (mask2 == mask2[:1, :]).all()
    )
    if structured:
        k1 = np.flatnonzero(mask1[:, 0])
        k2 = np.flatnonzero(mask2[0, :])
        kb = np.intersect1d(k1, k2)
        if kb.size == 0:
            return np.zeros((B, S, D), np.float32)
        W1 = w1[kb]             # [Hk, D], mask1 rows are all-ones here
        W2T = w2[:, kb].T       # [Hk, D], mask2 cols are all-ones here
    else:
        W1 = w1 * mask1
        W2T = (w2 * mask2).T
        if mode in ("f32r", "f32"):
            mode = "f16"        # dense fallback: halve SBUF footprint
            npdt = _NPDT[mode]
    Hk = W1.shape[0]
    Hg = max(P, ((Hk + P - 1) // P) * P)

    four_byte = mode in ("f32r", "f32")
    w1d = _pack_w1(W1, Hg, 3 if four_byte else 4, npdt)
    w2d = _pack_w2(W2T, Hg, npdt)

    # Token-parallel over cores, in sequential rounds if a full token
    # slice per core would not divide into TS chunks or not fit in SBUF
    # (x + hT are SBUF-resident: (KD + JT) * Tc * itemsize per partition).
    assert T % N_CORES == 0
    Tc = T // N_CORES
    itemsz = np.dtype(npdt).itemsize
    rounds = 1
    while (Tc // rounds) % TS != 0 or (Tc // rounds) == 0 or (
        (D // P + Hg // P) * (Tc // rounds) * itemsz > 140 * 1024
    ):
        rounds *= 2
        assert rounds <= 16, "input too large for SBUF tiling scheme"
    Tc //= rounds

    nc = _get_nc(D, Hg, Tc, mode)

    out = np.empty((T, D), np.float32)
    global LAST_RESULT
    for r in range(rounds):
        in_maps = []
        for c in range(N_CORES):
            t0 = (r * N_CORES + c) * Tc
            in_maps.append({
                "xt": _pack_x(x2[t0:t0 + Tc], npdt),
                "w1d": w1d,
                "w2d": w2d,
            })
        res = run_bass_kernel_spmd(
            nc, in_maps, core_ids=list(range(N_CORES)), trace=_trace,
        )
        LAST_RESULT = res
        for c in range(N_CORES):
            t0 = (r * N_CORES + c) * Tc
            o = res.results[c]["outt"]  # [NT, P, Tc] in device dtype
            out[t0:t0 + Tc] = o.reshape(D, Tc).T.astype(np.float32)

    return out.reshape(B, S, D)
